# revision 18
# baseline (speedup 1.0000x reference)
"""Trainium2 Bass kernel for the NeuralODE problem.

Math (matching reference.py):
    20 Euler steps (10 segments x 2 steps, uniform dt => step size hi = 0.05):
        z_{i+1} = z_i + hi * ( tanh(z_i @ W1 + b1 + t_i*wt) @ W2 + b2 )

Device-side reformulation (per core, batch shard B=64):
    - Fold hi into W2:  W2' = hi * W2, c = hi * b2.
    - Keep the "state without accumulated c":  z'_i = z_i - i*c, so
        z'_{i+1} = z'_i + tanh(z'_i @ W1 + bias_i) @ W2'
      with bias_i = b1 + t_i*wt + i*(c @ W1)   (precomputed on host).
      Final output: z_20 = z'_20 + 20*c       (added on host).
    - State kept transposed (d-major) as zT[p, 64k+b] = z'[b, 128k+p] so it can be
      the stationary (lhsT) operand of orientation-B matmuls.
    - Both matmuls stream the (SBUF-resident) weights as the moving operand with
      N=512 chunks; the 64-wide batch stationary only fills half the PE columns,
      so two chunks run concurrently via tile_position col-tiling (0,0)/(0,64).
    - The per-step bias enters PSUM first through a K=1 ones-vector matmul.
    - Layout flips (batch-major PSUM result -> d/hid-major stationary for the next
      matmul) are done with full-128 PE transpose-mode matmuls against identity;
      one 128x128 transpose covers one 128-col block of both concurrent chunks.
    - mm dtype parametric: float32 (exact, 4 cyc/row), float32r (1 cyc/row at
      N>=512, ~1e-4), bfloat16 (1 cyc/row, state kept fp32 + per-step snapshot).

Sharding: pure data-parallel over batch (512 -> 8 x 64); weights replicated.
"""

import numpy as np

BS, D, HID = 512, 1024, 2048
NCORES = 8
B = BS // NCORES  # 64
NSTEP = 20
KD = D // 128  # 8 k-tiles for the D contraction
KH = HID // 128  # 16 k-tiles for the HID contraction
F32 = np.float32

MM_DTYPE = "bfloat16"
H_TRANSPOSE = "dma"  # "dma" (XBAR, off-PE) or "pe" (transpose-mode matmuls)


def _np_dt(dt_name):
    if dt_name == "bfloat16":
        import ml_dtypes

        return ml_dtypes.bfloat16
    return np.float32


def _build_program(mm_dtype=MM_DTYPE, repeat=1):
    import concourse.mybir as mybir
    from concourse import bacc
    from concourse.tile import TileContext

    nc = bacc.Bacc()
    f32 = mybir.dt.float32
    mmdt = getattr(mybir.dt, mm_dtype)
    # dtype used for the fp32-ish state path. For float32r, declare state
    # tensors as float32r (same bits as fp32) to satisfy the BIR verifier's
    # "rounded to FP32r" producer rule; for bfloat16 the state stays fp32 and
    # a per-step bf16 snapshot feeds the matmuls.
    snapshot = mm_dtype == "bfloat16"
    stdt = f32 if snapshot else mmdt
    TANH = mybir.ActivationFunctionType.Tanh
    COPY = mybir.ActivationFunctionType.Copy

    zt_in = nc.dram_tensor("zt_in", [128, KD * B], stdt, kind="ExternalInput")
    w1_d = nc.dram_tensor("w1", [128, KD * HID], mmdt, kind="ExternalInput")
    w2_d = nc.dram_tensor("w2", [128, KH * D], mmdt, kind="ExternalInput")
    # bias2[r, 1024*i + 512*g + n] = bias_i[512*(2g+r) + n]; the K=2 selector
    # matmul sel2.T @ bias2-slice seeds both col-tile halves of a PSUM bank
    # with their bias rows in one instruction (single start=True per bank).
    biases_d = nc.dram_tensor("biases", [2, NSTEP * D], mmdt, kind="ExternalInput")
    ident_d = nc.dram_tensor("ident", [128, 128], mmdt, kind="ExternalInput")
    sel2_d = nc.dram_tensor("sel2", [2, 128], mmdt, kind="ExternalInput")
    zt_out = nc.dram_tensor("zt_out", [128, KD * B], stdt, kind="ExternalOutput")

    with (
        TileContext(nc) as tc,
        tc.tile_pool(name="const", bufs=1) as cpool,
        tc.tile_pool(name="weights", bufs=1) as wpool,
        tc.tile_pool(name="state", bufs=1) as spool,
        tc.tile_pool(name="work", bufs=2) as hpool,
        tc.tile_pool(name="psumh", bufs=2, space="PSUM") as ph_pool,
        tc.tile_pool(name="psumt", bufs=2, space="PSUM") as pt_pool,
        tc.tile_pool(name="psumf", bufs=2, space="PSUM") as pf_pool,
    ):
        # DMA issue order = availability order for step 0: selector, state,
        # biases, first W1 slice, identity, then the remaining weights.
        sel2_sb = cpool.tile([2, 128], mmdt, tag="sel2")
        nc.sync.dma_start(sel2_sb[:], sel2_d[:])

        zt = spool.tile([128, KD * B], stdt, tag="zt")  # z'_T  [128, 512]
        nc.sync.dma_start(zt[:], zt_in[:])
        if snapshot:
            zb = spool.tile([128, KD * B], mmdt, tag="zb")
        else:
            zb = zt
        hT = spool.tile([128, KH * B], mmdt, tag="hT")  # tanh'd h, hid-major [128,1024]

        # [2, .] tensors DMA at 2-partition bandwidth; split per step so step 0
        # only waits for its own 4KB slice, the rest land during compute.
        bias_sb = cpool.tile([2, NSTEP * D], mmdt, tag="bias")
        for i in range(NSTEP):
            nc.sync.dma_start(
                bias_sb[:, D * i : D * (i + 1)], biases_d[:, D * i : D * (i + 1)]
            )

        # per-k weight tiles so step-0 matmuls can start as soon as their
        # own k-slice has landed (whole-tensor deps would stall ~50us)
        w1t = []
        for k in range(KD):
            w = wpool.tile([128, HID], mmdt, tag=f"w1_{k}")
            nc.sync.dma_start(w[:], w1_d[:, k * HID : (k + 1) * HID])
            w1t.append(w)
            if k == 0:
                ident_sb = cpool.tile([128, 128], mmdt, tag="ident")
                nc.sync.dma_start(ident_sb[:], ident_d[:])
        w2t = []
        for k in range(KH):
            w = wpool.tile([128, D], mmdt, tag=f"w2_{k}")
            nc.sync.dma_start(w[:], w2_d[:, k * D : (k + 1) * D])
            w2t.append(w)

        if snapshot:
            nc.vector.tensor_copy(zb[:], zt[:])

        K_ORDER = list(range(KD))

        def scan_body(_iv=None):
            for i in range(NSTEP):
                # ---- mm1: h_pre = z @ W1 + bias_i, chunks of 512 over HID ----
                phs = []
                for g in range(2):
                    ph = ph_pool.tile([128, 512], f32, tag="ph")
                    phs.append(ph)
                    nc.tensor.matmul(
                        ph[:],
                        sel2_sb[:],
                        bias_sb[:, D * i + 512 * g : D * i + 512 * g + 512],
                        start=True,
                        stop=False,
                    )
                    for kidx, k in enumerate(K_ORDER):
                        for half in range(2):
                            c = 2 * g + half
                            nc.tensor.matmul(
                                ph[64 * half : 64 * half + 64, :],
                                zb[:, B * k : B * k + B],
                                w1t[k][:, 512 * c : 512 * c + 512],
                                start=False,
                                stop=(kidx == KD - 1),
                                tile_position=(0, 64 * half),
                            )

                # ---- tanh, then DMA-XBAR transpose to hid-major (off-PE) ----
                # dest block (g,u) = hT cols [128*(4g+u), +128) holds hid-blocks
                # j=8g+u (cols 0:64) and j=8g+4+u (cols 64:128) side by side.
                for g in range(2):
                    h_bm = hpool.tile([128, 512], mmdt, tag="h_bm")
                    nc.scalar.activation(h_bm[:], phs[g][:], TANH)
                    if H_TRANSPOSE == "dma":
                        for u in range(4):
                            nc.sync.dma_start(
                                hT[:, 128 * (4 * g + u) : 128 * (4 * g + u) + 128],
                                h_bm[:, 128 * u : 128 * u + 128],
                                transpose=True,
                            )
                    else:
                        pt = pt_pool.tile([128, 512], mmdt, tag="pt")
                        for u in range(4):
                            nc.tensor.matmul(
                                pt[:, 128 * u : 128 * u + 128],
                                h_bm[:, 128 * u : 128 * u + 128],
                                ident_sb[:],
                                is_transpose=True,
                                start=True,
                                stop=True,
                            )
                        nc.vector.tensor_copy(
                            hT[:, 512 * g : 512 * g + 512], pt[:]
                        )

                # ---- mm2: f' = h @ W2', chunks of 512 over D, col-tiled ----
                # hid-block j lives at 64-col slot pos(j) of hT (see above)
                pf = pf_pool.tile([128, 512], f32, tag="pf")
                for k in range(KH):
                    g_, r_ = k // 8, k % 8
                    pos = 8 * g_ + 2 * (r_ % 4) + r_ // 4
                    for half in range(2):
                        nc.tensor.matmul(
                            pf[64 * half : 64 * half + 64, :],
                            hT[:, B * pos : B * pos + B],
                            w2t[k][:, 512 * half : 512 * half + 512],
                            start=(k == 0),
                            stop=(k == KH - 1),
                            tile_position=(0, 64 * half),
                        )

                # ---- transpose f' to d-major and update state (split halves:
                # zb_next = bf16(zt_old + f) feeds mm1 first; the f32 zt
                # accumulation follows off the critical path) ----
                f_bm = hpool.tile([128, 512], mmdt, tag="f_bm")
                nc.scalar.activation(f_bm[:], pf[:], COPY)
                pt2 = pt_pool.tile([128, 512], mmdt, tag="pt")
                for u in range(4):
                    nc.tensor.matmul(
                        pt2[:, 128 * u : 128 * u + 128],
                        f_bm[:, 128 * u : 128 * u + 128],
                        ident_sb[:],
                        is_transpose=True,
                        start=True,
                        stop=True,
                    )
                zt_v = zt[:].rearrange("p (h u c) -> p h u c", h=2, u=4)
                pt2_v = pt2[:].rearrange("p (u h c) -> p h u c", u=4, h=2)
                if snapshot:
                    zb_v = zb[:].rearrange("p (h u c) -> p h u c", h=2, u=4)
                    nc.vector.tensor_add(zb_v, zt_v, pt2_v)
                nc.vector.tensor_add(zt_v, zt_v, pt2_v)

        if repeat == 1:
            scan_body()
        else:
            with tc.For_i(0, repeat, 1) as _i:
                scan_body(_i)

        nc.sync.dma_start(zt_out[:], zt[:])

    nc.compile()
    return nc


def _pack_zT(shard):  # [B, D] -> [128, KD*B]
    return np.ascontiguousarray(
        shard.T.reshape(KD, 128, B).transpose(1, 0, 2).reshape(128, KD * B)
    )


def _unpack_zT(zt):  # [128, KD*B] -> [B, D]
    return zt.reshape(128, KD, B).transpose(1, 0, 2).reshape(D, B).T


def _host_inputs(z0, t, W1, b1, wt, W2, b2):
    t = np.asarray(t, F32)
    t0s, t1s = t[:-1], t[1:]
    h_seg = (t1s - t0s) / 2.0  # N_STEPS_PER_SEG = 2
    step_ts = (t0s[:, None] + h_seg[:, None] * np.arange(2, dtype=F32)[None, :]).reshape(
        -1
    )
    step_hs = np.repeat(h_seg, 2)
    assert np.allclose(step_hs, step_hs[0]), "non-uniform Euler steps unsupported"
    scale = F32(step_hs[0])

    c = (scale * np.asarray(b2, F32)).astype(F32)  # [D]
    cW1 = (c.astype(np.float64) @ np.asarray(W1, np.float64)).astype(F32)  # [HID]
    biases = np.stack(
        [
            (np.asarray(b1, F32) + step_ts[i] * np.asarray(wt, F32) + i * cW1).astype(
                F32
            )
            for i in range(NSTEP)
        ]
    )  # [NSTEP, HID]
    # bias2[r, 1024*i + 512*g + n] = biases[i, 512*(2g+r) + n]
    bias2 = np.ascontiguousarray(
        biases.reshape(NSTEP, 2, 2, 512).transpose(2, 0, 1, 3).reshape(2, NSTEP * D)
    )
    sel2 = np.zeros((2, 128), F32)
    sel2[0, 0:64] = 1.0
    sel2[1, 64:128] = 1.0

    w1p = np.ascontiguousarray(
        np.asarray(W1, F32).reshape(KD, 128, HID).transpose(1, 0, 2).reshape(128, KD * HID)
    )
    w2p = np.ascontiguousarray(
        (scale * np.asarray(W2, F32))
        .astype(F32)
        .reshape(KH, 128, D)
        .transpose(1, 0, 2)
        .reshape(128, KH * D)
    )
    ident = np.eye(128, dtype=F32)
    return bias2, sel2, w1p, w2p, ident, c


def _make_in_maps(z0, t, W1, b1, wt, W2, b2, mm_dtype=MM_DTYPE):
    z0 = np.asarray(z0, F32)
    bias2, sel2, w1p, w2p, ident, c = _host_inputs(z0, t, W1, b1, wt, W2, b2)
    mdt = _np_dt(mm_dtype)
    in_maps = []
    for core in range(NCORES):
        shard = z0[core * B : (core + 1) * B]
        in_maps.append(
            {
                "zt_in": _pack_zT(shard),
                "w1": w1p.astype(mdt),
                "w2": w2p.astype(mdt),
                "biases": bias2.astype(mdt),
                "ident": ident.astype(mdt),
                "sel2": sel2.astype(mdt),
            }
        )
    return in_maps, c


def run(z0, t, W1, b1, wt, W2, b2, trace=False, mm_dtype=MM_DTYPE):
    from concourse.bass_utils import run_bass_kernel_spmd

    in_maps, c = _make_in_maps(z0, t, W1, b1, wt, W2, b2, mm_dtype=mm_dtype)
    nc = _build_program(mm_dtype=mm_dtype)
    res = run_bass_kernel_spmd(nc, in_maps, core_ids=list(range(NCORES)), trace=trace)

    outs = []
    for core in range(NCORES):
        z_shard = _unpack_zT(np.asarray(res.results[core]["zt_out"], F32))
        outs.append(z_shard)
    out = np.concatenate(outs, axis=0).astype(F32)
    out = out + (NSTEP * c)[None, :].astype(F32)
    return out.astype(F32), res


def kernel(z0, t, W1, b1, wt, W2, b2):
    out, _ = run(z0, t, W1, b1, wt, W2, b2, trace=False)
    return out


# revision 19
# speedup vs baseline: 1.4763x; 1.4763x over previous
"""Trainium2 Bass kernel for the NeuralODE problem.

Math (matching reference.py):
    20 Euler steps (10 segments x 2 steps, uniform dt => step size hi = 0.05):
        z_{i+1} = z_i + hi * ( tanh(z_i @ W1 + b1 + t_i*wt) @ W2 + b2 )

Device-side reformulation (per core, batch shard B=64):
    - Fold hi into W2:  W2' = hi * W2, c = hi * b2.
    - Keep the "state without accumulated c":  z'_i = z_i - i*c, so
        z'_{i+1} = z'_i + tanh(z'_i @ W1 + bias_i) @ W2'
      with bias_i = b1 + t_i*wt + i*(c @ W1)   (precomputed on host).
      Final output: z_20 = z'_20 + 20*c       (added on host).
    - State kept transposed (d-major) as zT[p, 64k+b] = z'[b, 128k+p] so it can be
      the stationary (lhsT) operand of orientation-B matmuls.
    - Both matmuls stream the (SBUF-resident) weights as the moving operand with
      N=512 chunks; the 64-wide batch stationary only fills half the PE columns,
      so two chunks run concurrently via tile_position col-tiling (0,0)/(0,64).
    - The per-step bias enters PSUM first through a K=1 ones-vector matmul.
    - Layout flips (batch-major PSUM result -> d/hid-major stationary for the next
      matmul) are done with full-128 PE transpose-mode matmuls against identity;
      one 128x128 transpose covers one 128-col block of both concurrent chunks.
    - mm dtype parametric: float32 (exact, 4 cyc/row), float32r (1 cyc/row at
      N>=512, ~1e-4), bfloat16 (1 cyc/row, state kept fp32 + per-step snapshot).

Sharding: pure data-parallel over batch (512 -> 8 x 64); weights replicated.
"""

import numpy as np

BS, D, HID = 512, 1024, 2048
NCORES = 8
B = BS // NCORES  # 64
NSTEP = 20
KD = D // 128  # 8 k-tiles for the D contraction
KH = HID // 128  # 16 k-tiles for the HID contraction
F32 = np.float32

MM_DTYPE = "bfloat16"
H_TRANSPOSE = "pe"  # "dma" (XBAR, off-PE) or "pe" (transpose-mode matmuls)


def _np_dt(dt_name):
    if dt_name == "bfloat16":
        import ml_dtypes

        return ml_dtypes.bfloat16
    return np.float32


def _build_program(mm_dtype=MM_DTYPE, repeat=1):
    import concourse.mybir as mybir
    from concourse import bacc
    from concourse.tile import TileContext

    nc = bacc.Bacc()
    f32 = mybir.dt.float32
    mmdt = getattr(mybir.dt, mm_dtype)
    # dtype used for the fp32-ish state path. For float32r, declare state
    # tensors as float32r (same bits as fp32) to satisfy the BIR verifier's
    # "rounded to FP32r" producer rule; for bfloat16 the state stays fp32 and
    # a per-step bf16 snapshot feeds the matmuls.
    snapshot = mm_dtype == "bfloat16"
    stdt = f32 if snapshot else mmdt
    TANH = mybir.ActivationFunctionType.Tanh
    COPY = mybir.ActivationFunctionType.Copy

    zt_in = nc.dram_tensor("zt_in", [128, KD * B], stdt, kind="ExternalInput")
    w1_d = nc.dram_tensor("w1", [128, KD * HID], mmdt, kind="ExternalInput")
    w2_d = nc.dram_tensor("w2", [128, KH * D], mmdt, kind="ExternalInput")
    # bias2[r, 1024*i + 512*g + n] = bias_i[512*(2g+r) + n]; the K=2 selector
    # matmul sel2.T @ bias2-slice seeds both col-tile halves of a PSUM bank
    # with their bias rows in one instruction (single start=True per bank).
    biases_d = nc.dram_tensor("biases", [2, NSTEP * D], mmdt, kind="ExternalInput")
    ident_d = nc.dram_tensor("ident", [128, 128], mmdt, kind="ExternalInput")
    sel2_d = nc.dram_tensor("sel2", [2, 128], mmdt, kind="ExternalInput")
    zt_out = nc.dram_tensor("zt_out", [128, KD * B], stdt, kind="ExternalOutput")

    with (
        TileContext(nc) as tc,
        tc.tile_pool(name="const", bufs=1) as cpool,
        tc.tile_pool(name="weights", bufs=1) as wpool,
        tc.tile_pool(name="state", bufs=1) as spool,
        tc.tile_pool(name="work", bufs=2) as hpool,
        tc.tile_pool(name="psumh", bufs=2, space="PSUM") as ph_pool,
        tc.tile_pool(name="psumt", bufs=2, space="PSUM") as pt_pool,
        tc.tile_pool(name="psumf", bufs=2, space="PSUM") as pf_pool,
    ):
        # DMA issue order = availability order for step 0: selector, state,
        # biases, first W1 slice, identity, then the remaining weights.
        sel2_sb = cpool.tile([2, 128], mmdt, tag="sel2")
        nc.sync.dma_start(sel2_sb[:], sel2_d[:])

        zt = spool.tile([128, KD * B], stdt, tag="zt")  # z'_T  [128, 512]
        nc.sync.dma_start(zt[:], zt_in[:])
        if snapshot:
            zb = spool.tile([128, KD * B], mmdt, tag="zb")
        else:
            zb = zt
        hT = spool.tile([128, KH * B], mmdt, tag="hT")  # tanh'd h, hid-major [128,1024]

        # [2, .] tensors DMA at 2-partition bandwidth; split per step so step 0
        # only waits for its own 4KB slice, the rest land during compute.
        bias_sb = cpool.tile([2, NSTEP * D], mmdt, tag="bias")
        for i in range(NSTEP):
            nc.sync.dma_start(
                bias_sb[:, D * i : D * (i + 1)], biases_d[:, D * i : D * (i + 1)]
            )

        # per-k weight tiles so step-0 matmuls can start as soon as their
        # own k-slice has landed (whole-tensor deps would stall ~50us)
        w1t = []
        for k in range(KD):
            w = wpool.tile([128, HID], mmdt, tag=f"w1_{k}")
            nc.sync.dma_start(w[:], w1_d[:, k * HID : (k + 1) * HID])
            w1t.append(w)
            if k == 0:
                ident_sb = cpool.tile([128, 128], mmdt, tag="ident")
                nc.sync.dma_start(ident_sb[:], ident_d[:])
        w2t = []
        for k in range(KH):
            w = wpool.tile([128, D], mmdt, tag=f"w2_{k}")
            nc.sync.dma_start(w[:], w2_d[:, k * D : (k + 1) * D])
            w2t.append(w)

        if snapshot:
            nc.vector.tensor_copy(zb[:], zt[:])

        K_ORDER = list(range(KD))

        def scan_body(_iv=None):
            for i in range(NSTEP):
                # ---- mm1: h_pre = z @ W1 + bias_i, chunks of 512 over HID ----
                phs = []
                for g in range(2):
                    ph = ph_pool.tile([128, 512], f32, tag="ph")
                    phs.append(ph)
                    nc.tensor.matmul(
                        ph[:],
                        sel2_sb[:],
                        bias_sb[:, D * i + 512 * g : D * i + 512 * g + 512],
                        start=True,
                        stop=False,
                    )
                    for kidx, k in enumerate(K_ORDER):
                        for half in range(2):
                            c = 2 * g + half
                            nc.tensor.matmul(
                                ph[64 * half : 64 * half + 64, :],
                                zb[:, B * k : B * k + B],
                                w1t[k][:, 512 * c : 512 * c + 512],
                                start=False,
                                stop=(kidx == KD - 1),
                                tile_position=(0, 64 * half),
                            )

                # ---- tanh, then DMA-XBAR transpose to hid-major (off-PE) ----
                # dest block (g,u) = hT cols [128*(4g+u), +128) holds hid-blocks
                # j=8g+u (cols 0:64) and j=8g+4+u (cols 64:128) side by side.
                for g in range(2):
                    h_bm = hpool.tile([128, 512], mmdt, tag="h_bm")
                    nc.scalar.activation(h_bm[:], phs[g][:], TANH)
                    if H_TRANSPOSE == "dma":
                        for u in range(4):
                            nc.sync.dma_start(
                                hT[:, 128 * (4 * g + u) : 128 * (4 * g + u) + 128],
                                h_bm[:, 128 * u : 128 * u + 128],
                                transpose=True,
                            )
                    else:
                        pt = pt_pool.tile([128, 512], mmdt, tag="pt")
                        for u in range(4):
                            nc.tensor.matmul(
                                pt[:, 128 * u : 128 * u + 128],
                                h_bm[:, 128 * u : 128 * u + 128],
                                ident_sb[:],
                                is_transpose=True,
                                start=True,
                                stop=True,
                            )
                        nc.vector.tensor_copy(
                            hT[:, 512 * g : 512 * g + 512], pt[:]
                        )

                # ---- mm2: f' = h @ W2', chunks of 512 over D, col-tiled ----
                # hid-block j lives at 64-col slot pos(j) of hT (see above)
                pf = pf_pool.tile([128, 512], f32, tag="pf")
                for k in range(KH):
                    g_, r_ = k // 8, k % 8
                    pos = 8 * g_ + 2 * (r_ % 4) + r_ // 4
                    for half in range(2):
                        nc.tensor.matmul(
                            pf[64 * half : 64 * half + 64, :],
                            hT[:, B * pos : B * pos + B],
                            w2t[k][:, 512 * half : 512 * half + 512],
                            start=(k == 0),
                            stop=(k == KH - 1),
                            tile_position=(0, 64 * half),
                        )

                # ---- transpose f' to d-major and update state (split halves:
                # zb_next = bf16(zt_old + f) feeds mm1 first; the f32 zt
                # accumulation follows off the critical path) ----
                f_bm = hpool.tile([128, 512], mmdt, tag="f_bm")
                nc.scalar.activation(f_bm[:], pf[:], COPY)
                pt2 = pt_pool.tile([128, 512], mmdt, tag="pt")
                for u in range(4):
                    nc.tensor.matmul(
                        pt2[:, 128 * u : 128 * u + 128],
                        f_bm[:, 128 * u : 128 * u + 128],
                        ident_sb[:],
                        is_transpose=True,
                        start=True,
                        stop=True,
                    )
                zt_v = zt[:].rearrange("p (h u c) -> p h u c", h=2, u=4)
                pt2_v = pt2[:].rearrange("p (u h c) -> p h u c", u=4, h=2)
                if snapshot:
                    zb_v = zb[:].rearrange("p (h u c) -> p h u c", h=2, u=4)
                    nc.vector.tensor_add(zb_v, zt_v, pt2_v)
                nc.vector.tensor_add(zt_v, zt_v, pt2_v)

        if repeat == 1:
            scan_body()
        else:
            with tc.For_i(0, repeat, 1) as _i:
                scan_body(_i)

        nc.sync.dma_start(zt_out[:], zt[:])

    nc.compile()
    return nc


def _pack_zT(shard):  # [B, D] -> [128, KD*B]
    return np.ascontiguousarray(
        shard.T.reshape(KD, 128, B).transpose(1, 0, 2).reshape(128, KD * B)
    )


def _unpack_zT(zt):  # [128, KD*B] -> [B, D]
    return zt.reshape(128, KD, B).transpose(1, 0, 2).reshape(D, B).T


def _host_inputs(z0, t, W1, b1, wt, W2, b2):
    t = np.asarray(t, F32)
    t0s, t1s = t[:-1], t[1:]
    h_seg = (t1s - t0s) / 2.0  # N_STEPS_PER_SEG = 2
    step_ts = (t0s[:, None] + h_seg[:, None] * np.arange(2, dtype=F32)[None, :]).reshape(
        -1
    )
    step_hs = np.repeat(h_seg, 2)
    assert np.allclose(step_hs, step_hs[0]), "non-uniform Euler steps unsupported"
    scale = F32(step_hs[0])

    c = (scale * np.asarray(b2, F32)).astype(F32)  # [D]
    cW1 = (c.astype(np.float64) @ np.asarray(W1, np.float64)).astype(F32)  # [HID]
    biases = np.stack(
        [
            (np.asarray(b1, F32) + step_ts[i] * np.asarray(wt, F32) + i * cW1).astype(
                F32
            )
            for i in range(NSTEP)
        ]
    )  # [NSTEP, HID]
    # bias2[r, 1024*i + 512*g + n] = biases[i, 512*(2g+r) + n]
    bias2 = np.ascontiguousarray(
        biases.reshape(NSTEP, 2, 2, 512).transpose(2, 0, 1, 3).reshape(2, NSTEP * D)
    )
    sel2 = np.zeros((2, 128), F32)
    sel2[0, 0:64] = 1.0
    sel2[1, 64:128] = 1.0

    w1p = np.ascontiguousarray(
        np.asarray(W1, F32).reshape(KD, 128, HID).transpose(1, 0, 2).reshape(128, KD * HID)
    )
    w2p = np.ascontiguousarray(
        (scale * np.asarray(W2, F32))
        .astype(F32)
        .reshape(KH, 128, D)
        .transpose(1, 0, 2)
        .reshape(128, KH * D)
    )
    ident = np.eye(128, dtype=F32)
    return bias2, sel2, w1p, w2p, ident, c


def _make_in_maps(z0, t, W1, b1, wt, W2, b2, mm_dtype=MM_DTYPE):
    z0 = np.asarray(z0, F32)
    bias2, sel2, w1p, w2p, ident, c = _host_inputs(z0, t, W1, b1, wt, W2, b2)
    mdt = _np_dt(mm_dtype)
    in_maps = []
    for core in range(NCORES):
        shard = z0[core * B : (core + 1) * B]
        in_maps.append(
            {
                "zt_in": _pack_zT(shard),
                "w1": w1p.astype(mdt),
                "w2": w2p.astype(mdt),
                "biases": bias2.astype(mdt),
                "ident": ident.astype(mdt),
                "sel2": sel2.astype(mdt),
            }
        )
    return in_maps, c


def run(z0, t, W1, b1, wt, W2, b2, trace=False, mm_dtype=MM_DTYPE):
    from concourse.bass_utils import run_bass_kernel_spmd

    in_maps, c = _make_in_maps(z0, t, W1, b1, wt, W2, b2, mm_dtype=mm_dtype)
    nc = _build_program(mm_dtype=mm_dtype)
    res = run_bass_kernel_spmd(nc, in_maps, core_ids=list(range(NCORES)), trace=trace)

    outs = []
    for core in range(NCORES):
        z_shard = _unpack_zT(np.asarray(res.results[core]["zt_out"], F32))
        outs.append(z_shard)
    out = np.concatenate(outs, axis=0).astype(F32)
    out = out + (NSTEP * c)[None, :].astype(F32)
    return out.astype(F32), res


def kernel(z0, t, W1, b1, wt, W2, b2):
    out, _ = run(z0, t, W1, b1, wt, W2, b2, trace=False)
    return out


# revision 23
# speedup vs baseline: 1.6172x; 1.0954x over previous
"""Trainium2 Bass kernel for the NeuralODE problem.

Math (matching reference.py):
    20 Euler steps (10 segments x 2 steps, uniform dt => step size hi = 0.05):
        z_{i+1} = z_i + hi * ( tanh(z_i @ W1 + b1 + t_i*wt) @ W2 + b2 )

Device-side reformulation (per core, batch shard B=64):
    - Fold hi into W2:  W2' = hi * W2, c = hi * b2.
    - Keep the "state without accumulated c":  z'_i = z_i - i*c, so
        z'_{i+1} = z'_i + tanh(z'_i @ W1 + bias_i) @ W2'
      with bias_i = b1 + t_i*wt + i*(c @ W1)   (precomputed on host).
      Final output: z_20 = z'_20 + 20*c       (added on host).
    - State kept transposed (d-major) as zT[p, 64k+b] = z'[b, 128k+p] so it can be
      the stationary (lhsT) operand of orientation-B matmuls.
    - Both matmuls stream the (SBUF-resident) weights as the moving operand with
      N=512 chunks; the 64-wide batch stationary only fills half the PE columns,
      so two chunks run concurrently via tile_position col-tiling (0,0)/(0,64).
    - The per-step bias enters PSUM first through a K=1 ones-vector matmul.
    - Layout flips (batch-major PSUM result -> d/hid-major stationary for the next
      matmul) are done with full-128 PE transpose-mode matmuls against identity;
      one 128x128 transpose covers one 128-col block of both concurrent chunks.
    - mm dtype parametric: float32 (exact, 4 cyc/row), float32r (1 cyc/row at
      N>=512, ~1e-4), bfloat16 (1 cyc/row, state kept fp32 + per-step snapshot).

Sharding: pure data-parallel over batch (512 -> 8 x 64); weights replicated.
"""

import numpy as np

BS, D, HID = 512, 1024, 2048
NCORES = 8
B = BS // NCORES  # 64
NSTEP = 20
KD = D // 128  # 8 k-tiles for the D contraction
KH = HID // 128  # 16 k-tiles for the HID contraction
F32 = np.float32

MM_DTYPE = "bfloat16"
H_TRANSPOSE = "pe"  # "dma" (XBAR, off-PE) or "pe" (transpose-mode matmuls)


def _np_dt(dt_name):
    if dt_name == "bfloat16":
        import ml_dtypes

        return ml_dtypes.bfloat16
    return np.float32


def _build_program(mm_dtype=MM_DTYPE, repeat=1):
    import concourse.mybir as mybir
    from concourse import bacc
    from concourse.tile import TileContext

    nc = bacc.Bacc()
    f32 = mybir.dt.float32
    mmdt = getattr(mybir.dt, mm_dtype)
    # dtype used for the fp32-ish state path. For float32r, declare state
    # tensors as float32r (same bits as fp32) to satisfy the BIR verifier's
    # "rounded to FP32r" producer rule; for bfloat16 the state stays fp32 and
    # a per-step bf16 snapshot feeds the matmuls.
    snapshot = mm_dtype == "bfloat16"
    stdt = f32 if snapshot else mmdt
    TANH = mybir.ActivationFunctionType.Tanh
    COPY = mybir.ActivationFunctionType.Copy

    zt_in = nc.dram_tensor("zt_in", [128, KD * B], stdt, kind="ExternalInput")
    w1_d = nc.dram_tensor("w1", [128, KD * HID], mmdt, kind="ExternalInput")
    w2_d = nc.dram_tensor("w2", [128, KH * D], mmdt, kind="ExternalInput")
    # bias2[r, 1024*i + 512*g + n] = bias_i[512*(2g+r) + n]; the K=2 selector
    # matmul sel2.T @ bias2-slice seeds both col-tile halves of a PSUM bank
    # with their bias rows in one instruction (single start=True per bank).
    biases_d = nc.dram_tensor("biases", [2, NSTEP * D], mmdt, kind="ExternalInput")
    ident_d = nc.dram_tensor("ident", [128, 128], mmdt, kind="ExternalInput")
    sel2_d = nc.dram_tensor("sel2", [2, 128], mmdt, kind="ExternalInput")
    zt_out = nc.dram_tensor("zt_out", [128, KD * B], stdt, kind="ExternalOutput")

    with (
        TileContext(nc) as tc,
        tc.tile_pool(name="const", bufs=1) as cpool,
        tc.tile_pool(name="weights", bufs=1) as wpool,
        tc.tile_pool(name="state", bufs=1) as spool,
        tc.tile_pool(name="work", bufs=2) as hpool,
        tc.tile_pool(name="psumh", bufs=2, space="PSUM") as ph_pool,
        tc.tile_pool(name="psumt", bufs=2, space="PSUM") as pt_pool,
        tc.tile_pool(name="psumth", bufs=1, space="PSUM") as pth_pool,
        tc.tile_pool(name="psumf", bufs=2, space="PSUM") as pf_pool,
    ):
        # DMA issue order = availability order for step 0: selector, state,
        # biases, first W1 slice, identity, then the remaining weights.
        sel2_sb = cpool.tile([2, 128], mmdt, tag="sel2")
        nc.sync.dma_start(sel2_sb[:], sel2_d[:])

        zt = spool.tile([128, KD * B], stdt, tag="zt")  # z'_T  [128, 512]
        nc.sync.dma_start(zt[:], zt_in[:])
        if snapshot:
            zb = spool.tile([128, KD * B], mmdt, tag="zb")
        else:
            zb = zt
        hT = spool.tile([128, KH * B], mmdt, tag="hT")  # tanh'd h, hid-major [128,1024]

        # [2, .] tensors DMA at 2-partition bandwidth; split per step so step 0
        # only waits for its own 4KB slice, the rest land during compute.
        bias_sb = cpool.tile([2, NSTEP * D], mmdt, tag="bias")
        for i in range(NSTEP):
            nc.sync.dma_start(
                bias_sb[:, D * i : D * (i + 1)], biases_d[:, D * i : D * (i + 1)]
            )

        # per-k weight tiles so step-0 matmuls can start as soon as their
        # own k-slice has landed (whole-tensor deps would stall ~50us)
        w1t = []
        for k in range(KD):
            w = wpool.tile([128, HID], mmdt, tag=f"w1_{k}")
            nc.sync.dma_start(w[:], w1_d[:, k * HID : (k + 1) * HID])
            w1t.append(w)
            if k == 0:
                ident_sb = cpool.tile([128, 128], mmdt, tag="ident")
                nc.sync.dma_start(ident_sb[:], ident_d[:])
        w2t = []
        for k in range(KH):
            w = wpool.tile([128, D], mmdt, tag=f"w2_{k}")
            nc.sync.dma_start(w[:], w2_d[:, k * D : (k + 1) * D])
            w2t.append(w)

        if snapshot:
            nc.vector.tensor_copy(zb[:], zt[:])

        # mm1 consumes zb d-blocks in the order the split f-tail refreshes
        # them (u-pair {0,1} covers blocks {0,1,4,5}, then {2,3,6,7})
        K_ORDER = [0, 1, 4, 5, 2, 3, 6, 7]

        def scan_body(_iv=None):
            for i in range(NSTEP):
                # ---- mm1: h_pre = z @ W1 + bias_i, chunks of 512 over HID ----
                phs = []
                for g in range(2):
                    ph = ph_pool.tile([128, 512], f32, tag="ph")
                    phs.append(ph)
                    nc.tensor.matmul(
                        ph[:],
                        sel2_sb[:],
                        bias_sb[:, D * i + 512 * g : D * i + 512 * g + 512],
                        start=True,
                        stop=False,
                    )
                    for kidx, k in enumerate(K_ORDER):
                        for half in range(2):
                            c = 2 * g + half
                            nc.tensor.matmul(
                                ph[64 * half : 64 * half + 64, :],
                                zb[:, B * k : B * k + B],
                                w1t[k][:, 512 * c : 512 * c + 512],
                                start=False,
                                stop=(kidx == KD - 1),
                                tile_position=(0, 64 * half),
                            )

                # ---- tanh, then DMA-XBAR transpose to hid-major (off-PE) ----
                # dest block (g,u) = hT cols [128*(4g+u), +128) holds hid-blocks
                # j=8g+u (cols 0:64) and j=8g+4+u (cols 64:128) side by side.
                for g in range(2):
                    h_bm = hpool.tile([128, 512], mmdt, tag="h_bm")
                    nc.scalar.activation(h_bm[:], phs[g][:], TANH)
                    if H_TRANSPOSE == "dma":
                        for u in range(4):
                            nc.sync.dma_start(
                                hT[:, 128 * (4 * g + u) : 128 * (4 * g + u) + 128],
                                h_bm[:, 128 * u : 128 * u + 128],
                                transpose=True,
                            )
                    else:
                        pt = pt_pool.tile([128, 512], mmdt, tag="pt")
                        for u in range(4):
                            nc.tensor.matmul(
                                pt[:, 128 * u : 128 * u + 128],
                                h_bm[:, 128 * u : 128 * u + 128],
                                ident_sb[:],
                                is_transpose=True,
                                start=True,
                                stop=True,
                            )
                        nc.vector.tensor_copy(
                            hT[:, 512 * g : 512 * g + 512], pt[:]
                        )

                # ---- mm2: f' = h @ W2', chunks of 512 over D, col-tiled ----
                # hid-block j lives at 64-col slot pos(j) of hT (see above)
                pf = pf_pool.tile([128, 512], f32, tag="pf")
                for k in range(KH):
                    g_, r_ = k // 8, k % 8
                    pos = 8 * g_ + 2 * (r_ % 4) + r_ // 4
                    for half in range(2):
                        nc.tensor.matmul(
                            pf[64 * half : 64 * half + 64, :],
                            hT[:, B * pos : B * pos + B],
                            w2t[k][:, 512 * half : 512 * half + 512],
                            start=(k == 0),
                            stop=(k == KH - 1),
                            tile_position=(0, 64 * half),
                        )

                # ---- transpose f' to d-major and update state (split halves:
                # zb_next = bf16(zt_old + f) feeds mm1 first; the f32 zt
                # accumulation follows off the critical path) ----
                # split halves in separate PSUM tensors (disjoint banks) so the
                # DVE adds on half A never touch the bank PE is transposing
                # into for half B; zb_next = bf16(zt_old + f) feeds mm1 first
                f_bm = hpool.tile([128, 512], mmdt, tag="f_bm")
                zt_v = zt[:].rearrange("p (h u c) -> p h u c", h=2, u=4)
                if snapshot:
                    zb_v = zb[:].rearrange("p (h u c) -> p h u c", h=2, u=4)
                pta0 = pth_pool.tile([128, 256], mmdt, tag="pta")
                pta1 = pth_pool.tile([128, 256], mmdt, tag="ptb")
                for hs, pta in enumerate((pta0, pta1)):
                    nc.scalar.activation(
                        f_bm[:, 256 * hs : 256 * hs + 256],
                        pf[:, 256 * hs : 256 * hs + 256],
                        COPY,
                    )
                    for j, u in enumerate((2 * hs, 2 * hs + 1)):
                        nc.tensor.matmul(
                            pta[:, 128 * j : 128 * j + 128],
                            f_bm[:, 128 * u : 128 * u + 128],
                            ident_sb[:],
                            is_transpose=True,
                            start=True,
                            stop=True,
                        )
                    zt_s = zt_v[:, :, 2 * hs : 2 * hs + 2, :]
                    pta_v = pta[:].rearrange("p (u h c) -> p h u c", u=2, h=2)
                    if snapshot:
                        nc.vector.tensor_add(
                            zb_v[:, :, 2 * hs : 2 * hs + 2, :], zt_s, pta_v
                        )
                    nc.vector.tensor_add(zt_s, zt_s, pta_v)

        if repeat == 1:
            scan_body()
        else:
            with tc.For_i(0, repeat, 1) as _i:
                scan_body(_i)

        nc.sync.dma_start(zt_out[:], zt[:])

    nc.compile()
    return nc


def _pack_zT(shard):  # [B, D] -> [128, KD*B]
    return np.ascontiguousarray(
        shard.T.reshape(KD, 128, B).transpose(1, 0, 2).reshape(128, KD * B)
    )


def _unpack_zT(zt):  # [128, KD*B] -> [B, D]
    return zt.reshape(128, KD, B).transpose(1, 0, 2).reshape(D, B).T


def _host_inputs(z0, t, W1, b1, wt, W2, b2):
    t = np.asarray(t, F32)
    t0s, t1s = t[:-1], t[1:]
    h_seg = (t1s - t0s) / 2.0  # N_STEPS_PER_SEG = 2
    step_ts = (t0s[:, None] + h_seg[:, None] * np.arange(2, dtype=F32)[None, :]).reshape(
        -1
    )
    step_hs = np.repeat(h_seg, 2)
    assert np.allclose(step_hs, step_hs[0]), "non-uniform Euler steps unsupported"
    scale = F32(step_hs[0])

    c = (scale * np.asarray(b2, F32)).astype(F32)  # [D]
    cW1 = (c.astype(np.float64) @ np.asarray(W1, np.float64)).astype(F32)  # [HID]
    biases = np.stack(
        [
            (np.asarray(b1, F32) + step_ts[i] * np.asarray(wt, F32) + i * cW1).astype(
                F32
            )
            for i in range(NSTEP)
        ]
    )  # [NSTEP, HID]
    # bias2[r, 1024*i + 512*g + n] = biases[i, 512*(2g+r) + n]
    bias2 = np.ascontiguousarray(
        biases.reshape(NSTEP, 2, 2, 512).transpose(2, 0, 1, 3).reshape(2, NSTEP * D)
    )
    sel2 = np.zeros((2, 128), F32)
    sel2[0, 0:64] = 1.0
    sel2[1, 64:128] = 1.0

    w1p = np.ascontiguousarray(
        np.asarray(W1, F32).reshape(KD, 128, HID).transpose(1, 0, 2).reshape(128, KD * HID)
    )
    w2p = np.ascontiguousarray(
        (scale * np.asarray(W2, F32))
        .astype(F32)
        .reshape(KH, 128, D)
        .transpose(1, 0, 2)
        .reshape(128, KH * D)
    )
    ident = np.eye(128, dtype=F32)
    return bias2, sel2, w1p, w2p, ident, c


def _make_in_maps(z0, t, W1, b1, wt, W2, b2, mm_dtype=MM_DTYPE):
    z0 = np.asarray(z0, F32)
    bias2, sel2, w1p, w2p, ident, c = _host_inputs(z0, t, W1, b1, wt, W2, b2)
    mdt = _np_dt(mm_dtype)
    in_maps = []
    for core in range(NCORES):
        shard = z0[core * B : (core + 1) * B]
        in_maps.append(
            {
                "zt_in": _pack_zT(shard),
                "w1": w1p.astype(mdt),
                "w2": w2p.astype(mdt),
                "biases": bias2.astype(mdt),
                "ident": ident.astype(mdt),
                "sel2": sel2.astype(mdt),
            }
        )
    return in_maps, c


def run(z0, t, W1, b1, wt, W2, b2, trace=False, mm_dtype=MM_DTYPE):
    from concourse.bass_utils import run_bass_kernel_spmd

    in_maps, c = _make_in_maps(z0, t, W1, b1, wt, W2, b2, mm_dtype=mm_dtype)
    nc = _build_program(mm_dtype=mm_dtype)
    res = run_bass_kernel_spmd(nc, in_maps, core_ids=list(range(NCORES)), trace=trace)

    outs = []
    for core in range(NCORES):
        z_shard = _unpack_zT(np.asarray(res.results[core]["zt_out"], F32))
        outs.append(z_shard)
    out = np.concatenate(outs, axis=0).astype(F32)
    out = out + (NSTEP * c)[None, :].astype(F32)
    return out.astype(F32), res


def kernel(z0, t, W1, b1, wt, W2, b2):
    out, _ = run(z0, t, W1, b1, wt, W2, b2, trace=False)
    return out
